# revision 1
# baseline (speedup 1.0000x reference)
"""Trainium2 Bass kernel for a 3-layer difflogic network (nn_Net_48610439856713).

Math: each layer o computes softmax(w[o])·ops16(a, b) with a = h[:, ia[o]],
b = h[:, ib[o]].  The 16 relaxed logic gates are all affine in {1, a, b, ab},
so the layer reduces to  h' = C0 + C1·a + C2·b + C3·a·b  with 4 per-neuron
coefficients derived on-device from softmax(w).

Sharding: 2 batch groups x 4 neuron shards over the 8 cores.  Core c handles
batch rows [(c//4)*256, ...) and neuron shard c%4 of every layer.  Activations
are bf16 in a transposed packed layout h^T[neuron, batch]; each layer's shard
outputs are exchanged with a 4-rank AllGather so every core holds the full
previous layer as its gather source.  Gathers use the SWDGE dma_gather
primitive (cost ~8ns/index of Q7 descriptor generation, the kernel's
bottleneck — which is why indices per core are minimized via neuron sharding).

Host-side bookkeeping is integer/layout only: slot permutations, index
relabeling through the packed layout, int16 index wrapping, weight-row
packing.  All float arithmetic (softmax, combine, sums) runs on device.
"""

import os
import numpy as np

P = 128
B = 512
BG = 2                  # batch groups
SH = 4                  # neuron shards
BC = B // BG            # 256 batch per core
IN = 193
NGROUP = 3
TAU = 100.0
N_CORES = 8

# layers 1/2: 16000 real neurons -> 4096 slots/shard (96 pads each)
NJ12 = 32               # j-columns per shard
REAL12 = 4000           # real neurons per shard
NS12 = NJ12 * P         # 4096 slots per shard
# layer 3: 15999 real -> 33 j-cols/shard; group g owns local j in [11g, 11g+11)
NJ3 = 33
JPG = 11                # j-cols per group per shard
NS3 = NJ3 * P           # 4224 slots per shard
SPG = 15999 // NGROUP   # 5333 real slots per group

_CACHE = {}


def _build_nc():
    import concourse.bacc as bacc
    import concourse.tile as tile
    import concourse.mybir as mybir

    f32 = mybir.dt.float32
    bf16 = mybir.dt.bfloat16
    i16 = mybir.dt.int16
    Alu = mybir.AluOpType
    Act = mybir.ActivationFunctionType
    Ax = mybir.AxisListType

    nc = bacc.Bacc("TRN2", target_bir_lowering=False, debug=False, num_devices=N_CORES)

    # ---- I/O ----
    xT = nc.dram_tensor("xT", [IN, BC], f32, kind="ExternalInput")
    wps = [
        nc.dram_tensor("w1p", [P, NJ12 * 16], f32, kind="ExternalInput"),
        nc.dram_tensor("w2p", [P, NJ12 * 16], f32, kind="ExternalInput"),
        nc.dram_tensor("w3p", [P, NJ3 * 16], f32, kind="ExternalInput"),
    ]
    idxs = []
    for l, ns in ((1, NS12), (2, NS12), (3, NS3)):
        # combined a+b index stream, chunk-interleaved: [a-chunk0 b-chunk0 ...]
        idxs.append(
            nc.dram_tensor(f"i{l}", [P, 2 * ns // 16], i16, kind="ExternalInput")
        )
    out_d = nc.dram_tensor("out", [1, NGROUP * BC], f32, kind="ExternalOutput")

    # collective buffers (h exchange, NCH j-chunks pipelined) and partial-sum
    # exchange.  g layout is chunk-major: row r = k*SH*P + s*P + p, unit
    # r*JCH + (j % JCH)  with JCH = NJ12//NCH j-cols per chunk.
    NCH_ = NCH
    JCH_ = JCH
    cins = [
        [
            nc.dram_tensor(f"cin{l}_{k}", [P, JCH * BC], bf16, kind="Internal")
            for k in range(NCH)
        ]
        for l in (1, 2)
    ]
    gs_ = [
        nc.dram_tensor("g1", [NCH * SH * P, JCH * BC], bf16, kind="Internal"),
        nc.dram_tensor("g2", [NCH * SH * P, JCH * BC], bf16, kind="Internal"),
    ]
    # warm-up collective: absorbs first-collective firmware latency while the
    # layer-1 gathers run.  Output is an (ignored) ExternalOutput so DCE keeps it.
    win = nc.dram_tensor("win", [P, 16], f32, kind="Internal")
    warm = nc.dram_tensor("warm", [SH * P, 16], f32, kind="Internal")
    pin = nc.dram_tensor("pin", [1, NGROUP * BC], f32, kind="Internal")
    pall = nc.dram_tensor("pall", [SH, NGROUP * BC], f32, kind="Internal")

    shard_groups = [[0, 1, 2, 3], [4, 5, 6, 7]]

    with tile.TileContext(nc) as tc:
        with (
            tc.tile_pool(name="big", bufs=1) as big,
            tc.tile_pool(name="prep", bufs=2) as prep,
            tc.tile_pool(name="small", bufs=2) as small,
            tc.tile_pool(name="psum", bufs=1, space="PSUM") as psum,
        ):
            layers = [
                (NJ12, NS12, f32, xT[:], idxs[0], wps[0], cins[0], gs_[0]),
                (
                    NJ12, NS12, bf16,
                    gs_[0][:].rearrange("r (j b) -> (r j) b", b=BC),
                    idxs[1], wps[1], cins[1], gs_[1],
                ),
                (
                    NJ3, NS3, bf16,
                    gs_[1][:].rearrange("r (j b) -> (r j) b", b=BC),
                    idxs[2], wps[2], None, None,
                ),
            ]  # cin entries are per-chunk lists for layers 1-2

            wsb = prep.tile([P, 16], f32, tag="wsb")
            nc.vector.memset(wsb[:], 0.0)
            nc.sync.dma_start(win[:], wsb[:])
            nc.gpsimd.collective_compute(
                "AllGather", Alu.bypass, replica_groups=shard_groups,
                ins=[win[:]], outs=[warm[:]],
            )

            h_final = None
            for li, (NJ, NS, gdt, src, iad, wp, cin, gout) in enumerate(layers):
                last = li == 2
                # ---- coefficient prep: C0..C3 [P, NJ] f32 ----
                wt = prep.tile([P, NJ * 16], f32, tag="wt")
                nc.sync.dma_start(wt[:], wp[:])
                e = prep.tile([P, NJ * 16], f32, tag="e")
                nc.scalar.activation(e[:], wt[:], Act.Exp)
                e3 = e[:].rearrange("p (j g) -> p j g", g=16)
                e4 = e[:].rearrange("p (j h q) -> p j h q", h=4, q=4)

                ssum = small.tile([P, NJ], f32, tag="ssum")
                nc.vector.reduce_sum(ssum[:], e3, axis=Ax.X)
                r = small.tile([P, NJ], f32, tag="r")
                nc.vector.reciprocal(r[:], ssum[:])

                c0 = small.tile([P, NJ], f32, tag="c0")
                c1 = small.tile([P, NJ], f32, tag="c1")
                c2 = small.tile([P, NJ], f32, tag="c2")
                c3 = small.tile([P, NJ], f32, tag="c3")

                nc.vector.reduce_sum(c0[:], e4[:, :, 2:4, :], axis=Ax.XY)
                t1 = small.tile([P, NJ], f32, tag="t1")
                t2 = small.tile([P, NJ], f32, tag="t2")
                nc.vector.reduce_sum(t1[:], e4[:, :, 0:2, 2:4], axis=Ax.XY)
                nc.vector.reduce_sum(t2[:], e4[:, :, 2:4, 0:2], axis=Ax.XY)
                nc.vector.tensor_sub(c1[:], t1[:], t2[:])
                t3 = small.tile([P, NJ], f32, tag="t3")
                t4 = small.tile([P, NJ], f32, tag="t4")
                nc.vector.reduce_sum(t3[:], e4[:, :, 1, :], axis=Ax.X)
                nc.vector.reduce_sum(t4[:], e4[:, :, 2, :], axis=Ax.X)
                nc.vector.tensor_sub(c2[:], t3[:], t4[:])
                f = small.tile([P, NJ, 7], f32, tag="f")
                nc.vector.tensor_sub(f[:], e3[:, :, 1:8], e3[:, :, 14:7:-1])
                u1 = small.tile([P, NJ], f32, tag="u1")
                u2 = small.tile([P, NJ], f32, tag="u2")
                nc.vector.tensor_sub(u1[:], f[:, :, 0], f[:, :, 1])
                nc.vector.tensor_add(u2[:], f[:, :, 3], f[:, :, 6])
                nc.vector.tensor_sub(u1[:], u1[:], u2[:])
                nc.vector.scalar_tensor_tensor(
                    c3[:], f[:, :, 5], -2.0, u1[:], op0=Alu.mult, op1=Alu.add
                )
                for ck in (c0, c1, c2, c3):
                    nc.vector.tensor_mul(ck[:], ck[:], r[:])

                # ---- idx load ----
                iab = prep.tile([P, 2 * NS // 16], i16, tag="iab")
                nc.sync.dma_start(iab[:], iad[:])

                # ---- chunked gathers + combine ----
                h = big.tile([P, NJ * BC], bf16, tag="h")
                h3 = h[:].rearrange("p (j b) -> p j b", b=BC)
                if last:
                    # group-aligned chunks so GroupSum reduces fire per chunk
                    chunks = [(0, 11), (11, 22), (22, NJ)]
                else:
                    chunks = [(k * JCH, (k + 1) * JCH) for k in range(NCH)]
                for ci, (j0, j1) in enumerate(chunks):
                    cw = j1 - j0
                    ab = big.tile([P, 2 * cw, BC], gdt, tag=f"ab{ci}")
                    nsc = 2 * cw * P
                    nc.gpsimd.dma_gather(
                        ab[:], src, iab[:, 2 * j0 * 8 : 2 * j1 * 8], nsc, nsc, BC,
                        single_packet=False,
                    )
                    tmp = big.tile([P, cw, BC], gdt, tag=f"t{ci}")
                    for j in range(j0, j1):
                        jl = j - j0
                        aj = ab[:, jl]
                        bj = ab[:, cw + jl]
                        # tmp = (a*C3)*b ; tmp = (a*C1)+tmp ; tmp = (b*C2)+tmp
                        nc.vector.scalar_tensor_tensor(
                            tmp[:, jl], aj, c3[:, j : j + 1], bj,
                            op0=Alu.mult, op1=Alu.mult,
                        )
                        nc.vector.scalar_tensor_tensor(
                            tmp[:, jl], aj, c1[:, j : j + 1], tmp[:, jl],
                            op0=Alu.mult, op1=Alu.add,
                        )
                        nc.vector.scalar_tensor_tensor(
                            tmp[:, jl], bj, c2[:, j : j + 1], tmp[:, jl],
                            op0=Alu.mult, op1=Alu.add,
                        )
                        # h = tmp + C0 on the Scalar engine
                        nc.scalar.activation(
                            h3[:, j], tmp[:, jl], Act.Identity,
                            bias=c0[:, j : j + 1], scale=1.0,
                        )

                    if not last:
                        # ship this chunk as soon as it's combined
                        nc.sync.dma_start(
                            cin[ci][:], h[:, j0 * BC : j1 * BC]
                        )
                        nc.gpsimd.collective_compute(
                            "AllGather", Alu.bypass, replica_groups=shard_groups,
                            ins=[cin[ci][:]],
                            outs=[gout[ci * SH * P : (ci + 1) * SH * P, :]],
                        )
                if last:
                    h_final = h

            # ---- GroupSum: per-shard partials, then cross-shard AllGather+sum ----
            gs = prep.tile([P, NGROUP * BC], f32, tag="gs")
            for g in range(NGROUP):
                sl = h_final[:, g * JPG * BC : (g + 1) * JPG * BC].rearrange(
                    "p (j b) -> p b j", b=BC
                )
                nc.vector.reduce_sum(gs[:, g * BC : (g + 1) * BC], sl, axis=Ax.X)
            ones = prep.tile([P, 1], f32, tag="ones")
            nc.vector.memset(ones[:], 1.0)
            psc = prep.tile([1, NGROUP * BC], f32, tag="psc")
            HW = NGROUP * BC // 2
            for k in range(2):
                ps = psum.tile([1, HW], f32, tag=f"ps{k}")
                nc.tensor.matmul(
                    ps[:], ones[:], gs[:, k * HW : (k + 1) * HW],
                    start=True, stop=True,
                )
                nc.scalar.copy(psc[:, k * HW : (k + 1) * HW], ps[:])
            nc.sync.dma_start(pin[:], psc[:])
            nc.gpsimd.collective_compute(
                "AllGather", Alu.bypass, replica_groups=shard_groups,
                ins=[pin[:]], outs=[pall[:]],
            )
            pall_sb = prep.tile([SH, NGROUP * BC], f32, tag="pall_sb")
            nc.sync.dma_start(pall_sb[:], pall[:])
            ones4 = prep.tile([SH, 1], f32, tag="ones4")
            nc.vector.memset(ones4[:], 1.0)
            osb = prep.tile([1, NGROUP * BC], f32, tag="osb")
            for k in range(2):
                ps2 = psum.tile([1, HW], f32, tag=f"ps2{k}")
                nc.tensor.matmul(
                    ps2[:], ones4[:], pall_sb[:, k * HW : (k + 1) * HW],
                    start=True, stop=True,
                )
                nc.scalar.mul(osb[:, k * HW : (k + 1) * HW], ps2[:], 1.0 / TAU)
            # consume the warm-up collective's (all-zero) output so DCE keeps it
            wsb2 = prep.tile([1, 16], f32, tag="wsb2")
            nc.sync.dma_start(wsb2[:], warm[0:1, :])
            nc.vector.tensor_add(osb[:, :16], osb[:, :16], wsb2[:])
            nc.sync.dma_start(out_d[:], osb[:])

    nc.compile()
    return nc


def _wrap_idx(ii):
    w = ii.astype(np.int16).reshape(-1, 16).T
    return np.ascontiguousarray(np.tile(w, (8, 1)))


CHUNKS12 = [(0, 8), (8, 16), (16, 24), (24, 32)]
CHUNKS3 = [(0, 11), (11, 22), (22, 33)]


def _combine_idx(ia_eff, ib_eff, chunk_list):
    """Interleave a/b index streams per chunk: [a-chunk0, b-chunk0, a-chunk1, ...]"""
    parts = []
    for j0, j1 in chunk_list:
        parts.append(ia_eff[j0 * P : j1 * P])
        parts.append(ib_eff[j0 * P : j1 * P])
    return _wrap_idx(np.concatenate(parts))


def _pack_w(w_eff, nj):
    # local slot t = j*128 + p  ->  packed[p, j*16+g]
    return np.ascontiguousarray(
        w_eff.reshape(nj, P, 16).transpose(1, 0, 2).reshape(P, nj * 16)
    )


NCH = 4
JCH = NJ12 // NCH


def _src_unit12(i):
    """BC-row unit of layer-1/2 neuron i in the chunk-major AllGathered
    [NCH*SH*128, JCH*BC] layout: shard s = i//4000, local t = i - 4000s,
    p = t%128, j = t//128, chunk k = j//JCH; row = (k*SH+s)*128+p,
    unit = row*JCH + j%JCH."""
    s = i // REAL12
    t = i - s * REAL12
    p = t % P
    j = t // P
    k = j // JCH
    return ((k * SH + s) * P + p) * JCH + j % JCH


def _host_pack(inputs):
    x = np.asarray(inputs["x"], dtype=np.float32)
    w1 = np.asarray(inputs["w1"], dtype=np.float32)
    w2 = np.asarray(inputs["w2"], dtype=np.float32)
    w3 = np.asarray(inputs["w3"], dtype=np.float32)
    i1a = np.asarray(inputs["idx1a"]).astype(np.int64)
    i1b = np.asarray(inputs["idx1b"]).astype(np.int64)
    i2a = np.asarray(inputs["idx2a"]).astype(np.int64)
    i2b = np.asarray(inputs["idx2b"]).astype(np.int64)
    i3a = np.asarray(inputs["idx3a"]).astype(np.int64)
    i3b = np.asarray(inputs["idx3b"]).astype(np.int64)

    pad_row = np.full(16, -20.0, dtype=np.float32)
    pad_row[0] = 20.0  # softmax -> ~one-hot FALSE gate -> h = 0

    per_shard = [dict() for _ in range(SH)]
    # layers 1 and 2: shard s owns real neurons [s*4000, (s+1)*4000)
    for l, (w, ja, jb, srcf) in enumerate(
        (
            (w1, i1a, i1b, lambda i: i),
            (w2, i2a, i2b, _src_unit12),
        ),
        start=1,
    ):
        for s in range(SH):
            sel = slice(s * REAL12, (s + 1) * REAL12)
            w_eff = np.concatenate(
                [w[sel], np.tile(pad_row, (NS12 - REAL12, 1))], axis=0
            )
            ia_eff = np.zeros(NS12, dtype=np.int64)
            ib_eff = np.zeros(NS12, dtype=np.int64)
            ia_eff[:REAL12] = srcf(ja[sel])
            ib_eff[:REAL12] = srcf(jb[sel])
            per_shard[s][f"w{l}p"] = _pack_w(w_eff, NJ12)
            per_shard[s][f"i{l}"] = _combine_idx(ia_eff, ib_eff, CHUNKS12)

    # layer 3: group g's 5333 real neurons split over shards as
    # counts c_s = [1334, 1333, 1333, 1333]; within (s, g): local j in
    # [11g, 11g+11), rank m = (j-11g)*128 + p
    counts = np.array([1334, 1333, 1333, 1333])
    offs = np.concatenate([[0], np.cumsum(counts)[:-1]])
    u = np.arange(NS3)
    jj = u // P
    pp = u % P
    gg = jj // JPG
    m = (jj - gg * JPG) * P + pp
    for s in range(SH):
        real = m < counts[s]
        rid = gg * SPG + offs[s] + np.minimum(m, counts[s] - 1)
        w3_eff = w3[rid].copy()
        w3_eff[~real] = pad_row
        i3a_eff = np.where(real, _src_unit12(i3a[rid]), 0)
        i3b_eff = np.where(real, _src_unit12(i3b[rid]), 0)
        per_shard[s]["w3p"] = _pack_w(w3_eff, NJ3)
        per_shard[s]["i3"] = _combine_idx(i3a_eff, i3b_eff, CHUNKS3)

    in_maps = []
    for c in range(N_CORES):
        G, s = c // SH, c % SH
        m_ = dict(per_shard[s])
        m_["xT"] = np.ascontiguousarray(x[G * BC : (G + 1) * BC].T)
        in_maps.append(m_)
    return in_maps


LAST_RESULTS = None


def kernel(**inputs):
    global LAST_RESULTS
    from concourse.bass_utils import run_bass_kernel_spmd

    if "nc" not in _CACHE:
        _CACHE["nc"] = _build_nc()
    nc = _CACHE["nc"]

    in_maps = _host_pack(inputs)
    trace = bool(int(os.environ.get("KERNEL_TRACE", "0")))
    res = run_bass_kernel_spmd(
        nc, in_maps, core_ids=list(range(N_CORES)), trace=trace
    )
    LAST_RESULTS = res

    out = np.empty((B, NGROUP), dtype=np.float32)
    for g_ in range(BG):
        rc = res.results[g_ * SH]["out"].reshape(NGROUP, BC)
        out[g_ * BC : (g_ + 1) * BC, :] = rc.T
    return out



# revision 11
# speedup vs baseline: 1.4513x; 1.4513x over previous
"""Trainium2 Bass kernel for a 3-layer difflogic network (nn_Net_48610439856713).

Math: each layer o computes softmax(w[o])·ops16(a, b) with a = h[:, ia[o]],
b = h[:, ib[o]].  The 16 relaxed logic gates are affine in {1, a, b, ab},
so a layer is h' = C0 + C1·a + C2·b + C3·a·b with per-neuron coefficients
derived on-device from softmax(w).

Design (v2):
 - 8 cores = 2 batch groups x 4 neuron shards.  Activations fp8(e4m3)
   everywhere (final GroupSum averages ~5333 neurons, so quantization noise
   cancels; validated ~3e-4 final rel err vs 2e-2 budget).
 - a-stream gathers run on the Tensor engine as one-hot matmuls: each layer's
   outputs are assigned to the core that owns their a-source shard and sorted
   by source slot, so each 128-output tile reads only NB=4 source j-columns.
   One-hot lhsT tiles are host-packed fp8 {0,1} inputs.  Layer 1 (source x,
   193 rows ~ 2 K-blocks) runs both streams on PE with no sorting.
 - b-stream gathers (random remote rows) use SWDGE dma_gather from the
   AllGathered fp8 activation buffer; descriptor generation is hoisted off
   the critical path with prepare_only + trigger_dma on 2 SWDGE queues.
 - Combine runs as 7 full-chunk DVE passes using stride-0 (broadcast) APs for
   the per-neuron coefficients; PSUM a/b tiles are staged to SBUF fp8 by the
   Scalar engine in groups of 8 j-columns.
 - GroupSum = 0/1-mask matmuls on PE ([128,3] masks x h3 tiles into PSUM),
   then a tiny 4-rank AllGather + ones-matmul as in the baseline.

Host-side work is integer/layout only: slot permutations, one-hot placement
(bytes 0x00/0x38), index relabeling, int16 wrapping, weight-row packing.
All float arithmetic (softmax, combine, sums) runs on device.
"""

import os
import numpy as np
import ml_dtypes

FP8 = ml_dtypes.float8_e4m3fn

P = 128
B = 512
BG = 2                  # batch groups
SH = 4                  # neuron shards
BC = B // BG            # 256 batch per core
IN = 193
NGROUP = 3
TAU = 100.0
N_CORES = 8
NB = 4                  # source j-blocks per output tile (PE a-gather window)

NJ1, R1 = 32, 4000      # layer1: 4000 real rows/shard, 4096 slots
NS1 = NJ1 * P
NJ2 = 34
NS2 = NJ2 * P           # 4352 slots (binomial 4000+-55, 6.4 sigma margin)
NJ3 = 34
NS3 = NJ3 * P
NCH = 2                 # AllGather chunks per layer
JCH1, JCH2, JCH3 = NJ1 // NCH, NJ2 // NCH, NJ3 // NCH
L1_16000 = 16000
L3N = 15999
SPG = L3N // NGROUP     # 5333

_CACHE = {}


def _k0(jj, ns_src, nj_src, ns_out):
    kc = int(round((jj * P + P // 2) * ns_src / ns_out / P))
    return min(max(kc - NB // 2, 0), nj_src - NB)


def _build_nc():
    import concourse.bacc as bacc
    import concourse.tile as tile
    import concourse.mybir as mybir

    f32 = mybir.dt.float32
    bf16 = mybir.dt.bfloat16
    fp8 = mybir.dt.float8e4
    i16 = mybir.dt.int16
    Alu = mybir.AluOpType
    Act = mybir.ActivationFunctionType
    Ax = mybir.AxisListType

    nc = bacc.Bacc(
        "TRN2", target_bir_lowering=False, debug=False, num_devices=N_CORES,
        num_swdge_queues=2,
    )

    # ---- I/O ----
    xT = nc.dram_tensor("xT", [IN, BC], f32, kind="ExternalInput")
    w1p = nc.dram_tensor("w1p", [P, NJ1 * 16], f32, kind="ExternalInput")
    w2p = nc.dram_tensor("w2p", [P, NJ2 * 16], f32, kind="ExternalInput")
    w3p = nc.dram_tensor("w3p", [P, NJ3 * 16], f32, kind="ExternalInput")
    oh1a = nc.dram_tensor("oh1a", [P, NJ1 * 2 * P], fp8, kind="ExternalInput")
    oh1b = nc.dram_tensor("oh1b", [P, NJ1 * 2 * P], fp8, kind="ExternalInput")
    oh2 = nc.dram_tensor("oh2", [P, NJ2 * NB * P], fp8, kind="ExternalInput")
    oh3 = nc.dram_tensor("oh3", [P, NJ3 * NB * P], fp8, kind="ExternalInput")
    i2d = nc.dram_tensor("i2", [P, NS2 // 16], i16, kind="ExternalInput")
    i3d = nc.dram_tensor("i3", [P, NS3 // 16], i16, kind="ExternalInput")
    gmask = nc.dram_tensor("gmask", [P, NJ3 * NGROUP], fp8, kind="ExternalInput")
    out_d = nc.dram_tensor("out", [1, NGROUP * BC], f32, kind="ExternalOutput")

    # collective buffers
    cin1 = [nc.dram_tensor(f"cin1_{k}", [P, JCH1 * BC], fp8, kind="Internal")
            for k in range(NCH)]
    cin2 = [nc.dram_tensor(f"cin2_{k}", [P, JCH2 * BC], fp8, kind="Internal")
            for k in range(NCH)]
    g1 = nc.dram_tensor("g1", [NCH * SH * P * JCH1, BC], fp8, kind="Internal")
    g2 = nc.dram_tensor("g2", [NCH * SH * P * JCH2, BC], fp8, kind="Internal")
    win = nc.dram_tensor("win", [P, 16], f32, kind="Internal")
    warm = nc.dram_tensor("warm", [SH * P, 16], f32, kind="Internal")
    pin = nc.dram_tensor("pin", [NGROUP, BC], f32, kind="Internal")
    pall = nc.dram_tensor("pall", [SH, NGROUP * BC], f32, kind="Internal")

    shard_groups = [[0, 1, 2, 3], [4, 5, 6, 7]]
    ROWS1 = SH * P * JCH1   # g1 rows per chunk
    ROWS2 = SH * P * JCH2

    with tile.TileContext(nc) as tc:
        with (
            tc.tile_pool(name="big", bufs=1) as big,
            tc.tile_pool(name="strm", bufs=2) as strm,
            tc.tile_pool(name="small", bufs=2) as small,
        ):
            # ---- warm-up collective ----
            wsb = small.tile([P, 16], f32, tag="wsb")
            nc.vector.memset(wsb[:], 0.0)
            nc.sync.dma_start(win[:], wsb[:])
            nc.gpsimd.collective_compute(
                "AllGather", Alu.bypass, replica_groups=shard_groups,
                ins=[win[:]], outs=[warm[:]],
            )

            # ---- x load + quantize ----
            xf = big.tile([P, 2, BC], f32, tag="xf")
            nc.vector.memset(xf[:], 0.0)
            nc.sync.dma_start(xf[:, 0], xT[0:P, :])
            nc.sync.dma_start(xf[0 : IN - P, 1], xT[P:IN, :])
            xq = big.tile([P, 2, BC], fp8, tag="xq")
            nc.scalar.copy(xq[:], xf[:])

            # ---- idx loads (prepare-only b-gathers are emitted after each
            # layer's AllGathers so the deferred RAW deps attach to the
            # trigger; the Pool engine still runs desc-gen early) ----
            i2sb = big.tile([P, NS2 // 16], i16, tag="i2sb")
            nc.sync.dma_start(i2sb[:], i2d[:])
            i3sb = big.tile([P, NS3 // 16], i16, tag="i3sb")
            nc.sync.dma_start(i3sb[:], i3d[:])
            b2 = big.tile([P, NJ2, BC], fp8, tag="b2")
            b3 = big.tile([P, NJ3, BC], fp8, tag="b3")
            sem2 = nc.alloc_semaphore("bdma2")
            sem3 = nc.alloc_semaphore("bdma3")

            # ---- coefficient prep (shared) ----
            def coeffs(wp, nj, lname):
                wt = strm.tile([P, nj * 16], f32, tag="wt")
                nc.sync.dma_start(wt[:], wp[:])
                e = strm.tile([P, nj * 16], f32, tag="e")
                nc.scalar.activation(e[:], wt[:], Act.Exp)
                e3 = e[:].rearrange("p (j g) -> p j g", g=16)
                e4 = e[:].rearrange("p (j h q) -> p j h q", h=4, q=4)
                ssum = small.tile([P, nj], f32, tag=f"ss{lname}")
                nc.vector.reduce_sum(ssum[:], e3, axis=Ax.X)
                r = small.tile([P, nj], f32, tag=f"r{lname}")
                nc.vector.reciprocal(r[:], ssum[:])
                c0 = small.tile([P, nj], f32, tag=f"c0{lname}")
                c1 = small.tile([P, nj], f32, tag=f"c1{lname}")
                c2 = small.tile([P, nj], f32, tag=f"c2{lname}")
                c3 = small.tile([P, nj], f32, tag=f"c3{lname}")
                nc.vector.reduce_sum(c0[:], e4[:, :, 2:4, :], axis=Ax.XY)
                t1 = small.tile([P, nj], f32, tag=f"t1{lname}")
                t2 = small.tile([P, nj], f32, tag=f"t2{lname}")
                nc.vector.reduce_sum(t1[:], e4[:, :, 0:2, 2:4], axis=Ax.XY)
                nc.vector.reduce_sum(t2[:], e4[:, :, 2:4, 0:2], axis=Ax.XY)
                nc.vector.tensor_sub(c1[:], t1[:], t2[:])
                nc.vector.reduce_sum(t1[:], e4[:, :, 1, :], axis=Ax.X)
                nc.vector.reduce_sum(t2[:], e4[:, :, 2, :], axis=Ax.X)
                nc.vector.tensor_sub(c2[:], t1[:], t2[:])
                f = small.tile([P, nj, 7], f32, tag=f"f{lname}")
                nc.vector.tensor_sub(f[:], e3[:, :, 1:8], e3[:, :, 14:7:-1])
                u1 = small.tile([P, nj], f32, tag=f"u1{lname}")
                u2 = small.tile([P, nj], f32, tag=f"u2{lname}")
                nc.vector.tensor_sub(u1[:], f[:, :, 0], f[:, :, 1])
                nc.vector.tensor_add(u2[:], f[:, :, 3], f[:, :, 6])
                nc.vector.tensor_sub(u1[:], u1[:], u2[:])
                nc.vector.scalar_tensor_tensor(
                    c3[:], f[:, :, 5], -2.0, u1[:], op0=Alu.mult, op1=Alu.add
                )
                for ck in (c0, c1, c2, c3):
                    nc.vector.tensor_mul(ck[:], ck[:], r[:])
                return c0, c1, c2, c3

            # ---- combine: h_cols = C0 + C1 a + C2 b + C3 ab over j-range ----
            JMAX = max(JCH1, JCH2, JCH3)

            def combine(h, a_sb, b_sb, cs, j0, j1, tag):
                c0, c1, c2, c3 = cs
                nj = j1 - j0
                tfull = strm.tile([P, JMAX, BC], bf16, tag="ct")
                ufull = strm.tile([P, JMAX, BC], bf16, tag="cu")
                t = tfull[:, :nj]
                u = ufull[:, :nj]

                def cb(c):
                    return c[:, j0:j1].unsqueeze(2).broadcast_to([P, nj, BC])

                av = a_sb[:, j0:j1]
                bv = b_sb[:, j0:j1]
                nc.vector.tensor_mul(t, av, bv)          # ab
                nc.vector.tensor_mul(t, t, cb(c3))       # C3 ab
                nc.vector.tensor_mul(u, av, cb(c1))      # C1 a
                nc.vector.tensor_add(t, t, u)
                nc.vector.tensor_mul(u, bv, cb(c2))      # C2 b
                nc.vector.tensor_add(t, t, u)
                nc.vector.tensor_add(h[:, j0:j1], t, cb(c0))

            # ================= LAYER 1 =================
            cs1 = coeffs(w1p, NJ1, "a")
            h1 = big.tile([P, NJ1, BC], fp8, tag="h1")
            a1 = big.tile([P, NJ1, BC], fp8, tag="a1")
            b1 = big.tile([P, NJ1, BC], fp8, tag="b1")

            with tc.tile_pool(name="psA", bufs=1, space="PSUM") as psA:
                # L1 PE: groups of 8 j, both streams
                for g0 in range(0, NJ1, 8):
                    oha = strm.tile([P, 8 * 2 * P], fp8, tag="oha")
                    ohb = strm.tile([P, 8 * 2 * P], fp8, tag="ohb")
                    nc.sync.dma_start(oha[:], oh1a[:, g0 * 2 * P : (g0 + 8) * 2 * P])
                    nc.sync.dma_start(ohb[:], oh1b[:, g0 * 2 * P : (g0 + 8) * 2 * P])
                    pa = psA.tile([P, 8, BC], f32, tag="pa")
                    pb = psA.tile([P, 8, BC], f32, tag="pb")
                    for jl in range(8):
                        for k in range(2):
                            nc.tensor.matmul(
                                pa[:, jl], oha[:, (jl * 2 + k) * P : (jl * 2 + k + 1) * P],
                                xq[:, k], start=(k == 0), stop=(k == 1),
                            )
                        for k in range(2):
                            nc.tensor.matmul(
                                pb[:, jl], ohb[:, (jl * 2 + k) * P : (jl * 2 + k + 1) * P],
                                xq[:, k], start=(k == 0), stop=(k == 1),
                            )
                    nc.scalar.copy(a1[:, g0 : g0 + 8], pa[:])
                    nc.scalar.copy(b1[:, g0 : g0 + 8], pb[:])

                # L1 combine + exchange per chunk
                for ch in range(NCH):
                    j0, j1 = ch * JCH1, (ch + 1) * JCH1
                    combine(h1, a1, b1, cs1, j0, j1, f"1{ch}")
                    nc.sync.dma_start(
                        cin1[ch][:],
                        h1[:, j0:j1].rearrange("p j b -> p (j b)"),
                    )
                    nc.gpsimd.collective_compute(
                        "AllGather", Alu.bypass, replica_groups=shard_groups,
                        ins=[cin1[ch][:]],
                        outs=[g1[ch * ROWS1 : (ch + 1) * ROWS1, :]],
                    )

                # prep b2 gather descriptors (fires via trigger later)
                nc.gpsimd.dma_gather(
                    b2[:], g1[:], i2sb[:], NS2, NS2, BC,
                    prepare_only=True, sem=sem2, queue_num=0, single_packet=False,
                )

                # ================= LAYER 2 =================
                cs2 = coeffs(w2p, NJ2, "b")
                h2 = big.tile([P, NJ2, BC], fp8, tag="h2")
                a2 = big.tile([P, NJ2, BC], fp8, tag="a2")

                # PE a-gather from local h1 (no exchange dependency)
                jgrps = [(g0, min(g0 + 8, NJ2)) for g0 in range(0, NJ2, 8)]
                for gi, (g0, g1e) in enumerate(jgrps):
                    gw = g1e - g0
                    ohg = strm.tile([P, 8 * NB * P], fp8, tag="ohg")
                    nc.sync.dma_start(
                        ohg[:, : gw * NB * P],
                        oh2[:, g0 * NB * P : g1e * NB * P],
                    )
                    pa = psA.tile([P, 8, BC], f32, tag="pa" if gi % 2 == 0 else "pb")
                    for jl in range(gw):
                        jj = g0 + jl
                        k0 = _k0(jj, NS1, NJ1, NS2)
                        for v in range(NB):
                            nc.tensor.matmul(
                                pa[:, jl],
                                ohg[:, (jl * NB + v) * P : (jl * NB + v + 1) * P],
                                h1[:, k0 + v],
                                start=(v == 0), stop=(v == NB - 1),
                            )
                    nc.scalar.copy(a2[:, g0:g1e], pa[:, :gw])

                # fire the b2 gather (waits on g1 writes via deferred deps)
                nc.gpsimd.trigger_dma(count=None, queue_num=0)

                for ch in range(NCH):
                    j0, j1 = ch * JCH2, (ch + 1) * JCH2
                    combine(h2, a2, b2, cs2, j0, j1, f"2{ch}")
                    nc.sync.dma_start(
                        cin2[ch][:],
                        h2[:, j0:j1].rearrange("p j b -> p (j b)"),
                    )
                    nc.gpsimd.collective_compute(
                        "AllGather", Alu.bypass, replica_groups=shard_groups,
                        ins=[cin2[ch][:]],
                        outs=[g2[ch * ROWS2 : (ch + 1) * ROWS2, :]],
                    )

                nc.gpsimd.dma_gather(
                    b3[:], g2[:], i3sb[:], NS3, NS3, BC,
                    prepare_only=True, sem=sem3, queue_num=1, single_packet=False,
                )

                # ================= LAYER 3 =================
                cs3 = coeffs(w3p, NJ3, "c")
                h3 = big.tile([P, NJ3, BC], fp8, tag="h3")
                a3 = big.tile([P, NJ3, BC], fp8, tag="a3")

                for gi, (g0, g1e) in enumerate(jgrps):
                    gw = g1e - g0
                    ohg = strm.tile([P, 8 * NB * P], fp8, tag="ohg")
                    nc.sync.dma_start(
                        ohg[:, : gw * NB * P],
                        oh3[:, g0 * NB * P : g1e * NB * P],
                    )
                    pa = psA.tile([P, 8, BC], f32, tag="pa" if gi % 2 == 0 else "pb")
                    for jl in range(gw):
                        jj = g0 + jl
                        k0 = _k0(jj, NS2, NJ2, NS3)
                        for v in range(NB):
                            nc.tensor.matmul(
                                pa[:, jl],
                                ohg[:, (jl * NB + v) * P : (jl * NB + v + 1) * P],
                                h2[:, k0 + v],
                                start=(v == 0), stop=(v == NB - 1),
                            )
                    nc.scalar.copy(a3[:, g0:g1e], pa[:, :gw])

                nc.gpsimd.trigger_dma(count=None, queue_num=1)

                for ch in range(NCH):
                    j0, j1 = ch * JCH3, (ch + 1) * JCH3
                    combine(h3, a3, b3, cs3, j0, j1, f"3{ch}")

            # ---- GroupSum: mask matmuls ----
            with tc.tile_pool(name="psB", bufs=1, space="PSUM") as psB:
                gm = big.tile([P, NJ3, NGROUP], fp8, tag="gm")
                nc.sync.dma_start(
                    gm[:], gmask[:].rearrange("p (j g) -> p j g", g=NGROUP)
                )
                pg = psB.tile([NGROUP, BC], f32, tag="pg")
                for j in range(NJ3):
                    nc.tensor.matmul(
                        pg[:], gm[:, j], h3[:, j],
                        start=(j == 0), stop=(j == NJ3 - 1),
                    )
                psc = small.tile([NGROUP, BC], f32, tag="psc")
                nc.scalar.copy(psc[:], pg[:])
                nc.sync.dma_start(pin[:], psc[:])
                nc.gpsimd.collective_compute(
                    "AllGather", Alu.bypass, replica_groups=shard_groups,
                    ins=[pin[:]], outs=[pall[:]],
                )
                pall_sb = small.tile([SH, NGROUP * BC], f32, tag="pall_sb")
                nc.sync.dma_start(pall_sb[:], pall[:])
                ones4 = small.tile([SH, 1], f32, tag="ones4")
                nc.vector.memset(ones4[:], 1.0)
                osb = small.tile([1, NGROUP * BC], f32, tag="osb")
                HW = NGROUP * BC // 2
                for k in range(2):
                    ps2 = psB.tile([1, HW], f32, tag=f"ps2{k}")
                    nc.tensor.matmul(
                        ps2[:], ones4[:], pall_sb[:, k * HW : (k + 1) * HW],
                        start=True, stop=True,
                    )
                    nc.scalar.mul(osb[:, k * HW : (k + 1) * HW], ps2[:], 1.0 / TAU)
                # consume the warm-up collective output so DCE keeps it
                wsb2 = small.tile([1, 16], f32, tag="wsb2")
                nc.sync.dma_start(wsb2[:], warm[0:1, :])
                nc.vector.tensor_add(osb[:, :16], osb[:, :16], wsb2[:])
                nc.sync.dma_start(out_d[:], osb[:])

    # ---- surgical sync rewiring for the prepare_only gathers ----
    # Tile places the g-buffer RAW dep (Collectives sem) on the prep itself
    # (blocking desc-gen until the AllGather lands) and leaves consumers
    # waiting on a DMASW lane sem that nothing increments (the SDMA completion
    # bumps the sem= baked into the descriptors instead).  Move the
    # Collectives waits prep -> trigger and repoint the orphaned DMASW waits
    # at the descriptor completion sems.
    import bass_rust

    insts = list(nc.all_instructions())
    preps = [
        i for i in insts
        if type(i).__name__ == "InstDMAGatherAnt" and getattr(i, "gen_mode", 0) == 1
    ]
    trigs = {
        i.queue_num: i for i in insts if type(i).__name__ == "InstTriggerDma"
    }
    assert len(preps) == 2 and len(trigs) == 2
    updated = set()
    for i in insts:
        if i.sync_info:
            for u in (i.sync_info.on_update or []):
                updated.add(u.ant_name)
    lane_sem = {}  # "DMASW<k>" -> descriptor completion SyncUpdate
    for li, p in enumerate(preps):
        dma_upd = p.sync_info.on_update[0]
        lane_sem[f"DMASW{li}"] = dma_upd
        keep, move = [], []
        for w in p.sync_info.on_wait or []:
            (move if w.ant_name.startswith("Collectives") else keep).append(w)
        p.sync_info.on_wait = keep
        t = trigs[p.queue_num]
        t.sync_info.on_wait = list(t.sync_info.on_wait or []) + move
    n_rewired = 0
    for i in insts:
        si = i.sync_info
        if not si or not si.on_wait:
            continue
        nw, changed = [], False
        for w in si.on_wait:
            base = w.ant_name.rsplit("_", 1)[0]
            if (
                w.ant_name.startswith("DMASW")
                and w.ant_name not in updated
                and base in lane_sem
            ):
                u = lane_sem[base]
                nw.append(bass_rust.SyncWait(
                    sync_type="semaphore", id=u.id, ant_name=u.ant_name,
                    wait_mode=w.wait_mode, wait_value=w.wait_value, wait_reg=None,
                ))
                changed = True
                n_rewired += 1
            else:
                nw.append(w)
        if changed:
            si.on_wait = nw
    assert n_rewired >= 2, f"expected orphan DMASW waits, rewired {n_rewired}"

    nc.compile()
    return nc


# ===================== host packing =====================

def _wrap_idx(ii):
    w = ii.astype(np.int16).reshape(-1, 16).T
    return np.ascontiguousarray(np.tile(w, (8, 1)))


def _pack_w(w_eff, nj):
    # slot t = j*128 + p  ->  packed[p, j*16+g]
    return np.ascontiguousarray(
        w_eff.reshape(nj, P, 16).transpose(1, 0, 2).reshape(P, nj * 16)
    )


PAD_ROW = np.full(16, -20.0, dtype=np.float32)
PAD_ROW[0] = 20.0  # softmax -> ~one-hot FALSE gate -> h = 0


def _assign_slots(la, ns_out):
    """Sort outputs by local a-slot, spread pads uniformly.
    la: per-output local source slot.  Returns slot index per output."""
    n = len(la)
    assert n <= ns_out, f"shard overflow: {n} > {ns_out}"
    order = np.argsort(la, kind="stable")
    slots = np.empty(n, dtype=np.int64)
    slots[order] = (np.arange(n, dtype=np.int64) * ns_out) // n
    return slots


def _onehot_pack(la_by_slot, ns_src, nj_src, ns_out):
    """Build [P, NT*NB*P] uint8 one-hot lhsT data (0x00 / 0x38==1.0 e4m3).
    la_by_slot: ns_out array, local source slot per output slot (-1 = pad)."""
    nt = ns_out // P
    oh = np.zeros((P, nt * NB * P), dtype=np.uint8)
    for jj in range(nt):
        k0 = _k0(jj, ns_src, nj_src, ns_out)
        for m in range(P):
            la = la_by_slot[jj * P + m]
            if la < 0:
                continue
            r = la - k0 * P
            v, rr = divmod(r, P)
            assert 0 <= v < NB, (
                f"window violation jj={jj} m={m} la={la} k0={k0}"
            )
            oh[rr, (jj * NB + v) * P + m] = 0x38
    return oh.view(FP8)


def _host_pack(inputs):
    x = np.asarray(inputs["x"], dtype=np.float32)
    w1 = np.asarray(inputs["w1"], dtype=np.float32)
    w2 = np.asarray(inputs["w2"], dtype=np.float32)
    w3 = np.asarray(inputs["w3"], dtype=np.float32)
    i1a = np.asarray(inputs["idx1a"]).astype(np.int64)
    i1b = np.asarray(inputs["idx1b"]).astype(np.int64)
    i2a = np.asarray(inputs["idx2a"]).astype(np.int64)
    i2b = np.asarray(inputs["idx2b"]).astype(np.int64)
    i3a = np.asarray(inputs["idx3a"]).astype(np.int64)
    i3b = np.asarray(inputs["idx3b"]).astype(np.int64)

    per_shard = [dict() for _ in range(SH)]

    # ---------- layer 1: contiguous assignment, no sorting ----------
    # slot of orig row i: shard i//4000, local t = i - 4000*shard
    for s in range(SH):
        sel = np.arange(s * R1, (s + 1) * R1)
        w_eff = np.concatenate(
            [w1[sel], np.tile(PAD_ROW, (NS1 - R1, 1))], axis=0
        )
        per_shard[s]["w1p"] = _pack_w(w_eff, NJ1)
        # one-hots: column m of tile jj selects x row ia1[orig]
        for nm, idx in (("oh1a", i1a), ("oh1b", i1b)):
            oh = np.zeros((P, NJ1 * 2 * P), dtype=np.uint8)
            for jj in range(NJ1):
                for m in range(P):
                    t = jj * P + m
                    if t >= R1:
                        continue
                    src = idx[s * R1 + t]
                    k, r = divmod(int(src), P)
                    oh[r, (jj * 2 + k) * P + m] = 0x38
            per_shard[s][nm] = oh.view(FP8)

    # ---------- layer 2 ----------
    s2 = i2a // R1                      # shard by a-source
    la2 = i2a - s2 * R1                 # local a-slot (layer-1 local t)
    slot2 = np.zeros(L1_16000, dtype=np.int64)   # orig o2 -> global slot
    for s in range(SH):
        sel = np.where(s2 == s)[0]
        sl = _assign_slots(la2[sel], NS2)
        slot2[sel] = s * NS2 + sl
        la_by_slot = np.full(NS2, -1, dtype=np.int64)
        la_by_slot[sl] = la2[sel]
        per_shard[s]["oh2"] = _onehot_pack(la_by_slot, NS1, NJ1, NS2)
        w_eff = np.tile(PAD_ROW, (NS2, 1))
        w_eff[sl] = w2[sel]
        per_shard[s]["w2p"] = _pack_w(w_eff, NJ2)
        # b-idx: g1 flat row of ib2 (layer-1 slot (sb, tb))
        ib = i2b[sel]
        sb, tb = ib // R1, ib % R1
        jb, pb = tb // P, tb % P
        ch = jb // JCH1
        row = ((ch * SH + sb) * P + pb) * JCH1 + (jb - ch * JCH1)
        idx_eff = np.zeros(NS2, dtype=np.int64)
        idx_eff[sl] = row
        per_shard[s]["i2"] = _wrap_idx(idx_eff)

    # ---------- layer 3 ----------
    g3 = slot2[i3a]
    s3 = g3 // NS2
    la3 = g3 - s3 * NS2
    grp = np.arange(L3N) // SPG
    for s in range(SH):
        sel = np.where(s3 == s)[0]
        sl = _assign_slots(la3[sel], NS3)
        la_by_slot = np.full(NS3, -1, dtype=np.int64)
        la_by_slot[sl] = la3[sel]
        per_shard[s]["oh3"] = _onehot_pack(la_by_slot, NS2, NJ2, NS3)
        w_eff = np.tile(PAD_ROW, (NS3, 1))
        w_eff[sl] = w3[sel]
        per_shard[s]["w3p"] = _pack_w(w_eff, NJ3)
        gb = slot2[i3b[sel]]
        sb, tb = gb // NS2, gb % NS2
        jb, pb = tb // P, tb % P
        ch = jb // JCH2
        row = ((ch * SH + sb) * P + pb) * JCH2 + (jb - ch * JCH2)
        idx_eff = np.zeros(NS3, dtype=np.int64)
        idx_eff[sl] = row
        per_shard[s]["i3"] = _wrap_idx(idx_eff)
        # group masks: [p, j*3+g] = 1.0 iff slot real and group(orig)==g
        gmask = np.zeros((NS3, NGROUP), dtype=np.uint8)
        gmask[sl, grp[sel]] = 0x38
        per_shard[s]["gmask"] = np.ascontiguousarray(
            gmask.reshape(NJ3, P, NGROUP).transpose(1, 0, 2).reshape(P, NJ3 * NGROUP)
        ).view(FP8)

    in_maps = []
    for c in range(N_CORES):
        G, s = c // SH, c % SH
        m_ = dict(per_shard[s])
        m_["xT"] = np.ascontiguousarray(x[G * BC : (G + 1) * BC].T)
        in_maps.append(m_)
    return in_maps


LAST_RESULTS = None


def kernel(**inputs):
    global LAST_RESULTS
    from concourse.bass_utils import run_bass_kernel_spmd

    if "nc" not in _CACHE:
        _CACHE["nc"] = _build_nc()
    nc = _CACHE["nc"]

    in_maps = _host_pack(inputs)
    trace = bool(int(os.environ.get("KERNEL_TRACE", "0")))
    res = run_bass_kernel_spmd(
        nc, in_maps, core_ids=list(range(N_CORES)), trace=trace
    )
    LAST_RESULTS = res

    out = np.empty((B, NGROUP), dtype=np.float32)
    for g_ in range(BG):
        rc = res.results[g_ * SH]["out"].reshape(NGROUP, BC)
        out[g_ * BC : (g_ + 1) * BC, :] = rc.T
    return out
